# revision 17
# baseline (speedup 1.0000x reference)
"""DA-RNN input-attention encoder kernel for Trainium2 (8 NeuronCores, SPMD).

Problem shapes (hardcoded): B=128, T=256, N=256, M=256.
Sharding: data-parallel over batch, 16 rows per core; weights replicated.

Key algebraic refactor (per reference):
  e[b,n,t'] = tanh( hs[b] @ WU_h[t']  +  X_perm[b,n] @ WU_x[t'] ) , then e @ ve
where WU_e = [WU_h | WU_x] split along its last dim (2M columns vs T columns).
  - C[b,n,t'] = X_perm[b,n] @ WU_x[t']  is step-invariant -> computed once.
  - A[b,t']   = hs[b] @ WU_h[t']        is tiny (rank-2M) -> per-step matmul.
Per step: P = tanh(C + A broadcast over n); e = P @ ve; softmax over n;
x_tilde = x_t * alpha; one LSTM step.

Device-side tricks:
  - kernel carries H2=2h, D=2c so sigmoid(x)=0.5*(1+tanh(x/2)) needs no
    affine; 0.5 factors folded into weights host-side.
  - C stored (t'-part, n-outer, b-inner) bf16 so the A broadcast-add is a
    b-contiguous bf16 DVE op (2x mode eligible).
  - e computed transposed (n on partitions) with P slices as stationary
    matmul operands; softmax sum via ones-matmul; 1/sum broadcast over gate
    partitions via an outer-product matmul (x_tilde never built).
  - gates/LSTM computed transposed ([gate, b] on partitions) so pointwise
    ops use all 128 lanes and h^T/c^T feed the next step without per-step
    PE transposes; full-f32 h/c kept for the recurrence, fp32r rounded
    copies feed the matmuls.
  - exp+tanh share one ACT table set; no other transcendentals used.
  - output h^T is PE-transposed back to [b, m] and row-quantized to int8
    with a per-(t,b) abs-max scale (q = 2h*126.5/mx, scl = mx/253, so
    h = q*scl exactly; quantization error <= 0.4% of each row's max).

Host-side dispatch (the wall-clock is transport-dominated -- the axon
tunnel has ~70 ms RTT and ~75 MB/s):
  - the jit(shard_map(bass_exec)) is AOT-compiled ONCE per process via
    fast_dispatch_compile (run_bass_kernel_spmd's axon path re-traces and
    re-lowers every call, which cost ~4.5 s/call).
  - weights are preprocessed + device_put once (crc32-keyed); X is
    device_put once and verified by crc32 AFTER submitting the execution
    speculatively (hash overlaps the remote exec; mismatch re-executes).
  - donated output buffers are recycled from the previous call's output
    (no host zero upload, no on-device zeros round trip).
  - int8 payload (8.4 MB) and f32 scales are fetched concurrently; host
    dequantizes into the final (T, B, M) f32 in one numpy pass.
"""

import os
import time
import zlib
from contextlib import ExitStack

import numpy as np

import concourse.bass as bass
from concourse import bacc
import concourse.mybir as mybir
import concourse.tile as tile
from concourse.bass_utils import run_bass_kernel_spmd

DEBUG_TIMING = bool(os.environ.get("KERNEL_DEBUG"))

B, T, N, M = 128, 256, 256, 256
NCORES = 8
BL = B // NCORES  # 16 batch rows per core
TSTEPS = int(os.environ.get("KERNEL_TSTEPS", str(T)))  # reduced-T for dev only
REPEAT = int(os.environ.get("KERNEL_REPEAT", "1"))  # timing isolation (dev only)
SKIP = set(x for x in os.environ.get("KERNEL_SKIP", "").split(",") if x)

F32 = mybir.dt.float32
F32R = mybir.dt.float32r
BF16 = mybir.dt.bfloat16
U16 = mybir.dt.uint16
AF = mybir.ActivationFunctionType
ALU = mybir.AluOpType


def _bc_ap(ap: bass.AP, offset_elems: int, dims) -> bass.AP:
    """Custom free-dim AP over the same tensor (steps in elements).

    Keeps the base AP's partition dim (its step is the per-partition pitch).
    `dims` are free dims only, outer->inner [step, count].
    """
    return bass.AP(
        tensor=ap.tensor, offset=ap.offset + offset_elems, ap=[ap.ap[0]] + list(dims)
    )


def build_program():
    nc = bacc.Bacc("TRN2", target_bir_lowering=False)

    X_d = nc.dram_tensor("X", (BL, T, N), F32, kind="ExternalInput")
    WUxT_d = nc.dram_tensor("WUxT", (T, T), F32, kind="ExternalInput")  # (j, t')
    WUhT_d = nc.dram_tensor("WUhT", (2 * M, T), F32, kind="ExternalInput")  # (d, t')
    WxT_d = nc.dram_tensor("WxT", (N, 4 * M), F32, kind="ExternalInput")  # (n, g)
    WhT_d = nc.dram_tensor("WhT", (M, 4 * M), F32, kind="ExternalInput")  # (m, g)
    bc_d = nc.dram_tensor("bc", (1, 4 * M), F32, kind="ExternalInput")
    ve_d = nc.dram_tensor("ve", (T, 1), F32, kind="ExternalInput")
    id_d = nc.dram_tensor("ident", (128, 128), F32, kind="ExternalInput")
    # output int8 row-quantized (per (t,b) abs-max scale) to cut D2H bytes;
    # host reconstructs h = q * scl (0.5 un-2h fold baked into scl)
    # payload split into NSPLIT tensors so D2H streams overlap on the tunnel
    NSPLIT = 8
    TQ = (TSTEPS + NSPLIT - 1) // NSPLIT
    outs_d = []
    for p in range(NSPLIT):
        tp = min(TQ, TSTEPS - p * TQ)
        if tp <= 0:
            break
        outs_d.append(
            nc.dram_tensor(
                f"out{p}", (tp, BL, M), mybir.dt.int8, kind="ExternalOutput"
            )
        )
    scl_d = nc.dram_tensor("scl", (TSTEPS, BL), F32, kind="ExternalOutput")

    with tile.TileContext(nc) as tc, ExitStack() as ctx:
        consts = ctx.enter_context(tc.tile_pool(name="consts", bufs=1))

        # ---- persistent weights in SBUF ----
        wuh_sb = consts.tile([128, 4 * T], F32, tag="wuh")
        for kt in range(4):
            nc.sync.dma_start(
                out=wuh_sb[:, kt * T : (kt + 1) * T],
                in_=WUhT_d[kt * 128 : (kt + 1) * 128, :],
            )
        wx_sb = consts.tile([128, 2 * 4 * M], F32R, tag="wx")
        wh_sb = consts.tile([128, 2 * 4 * M], F32R, tag="wh")
        bc_sb = consts.tile([1, 4 * M], F32R, tag="bc")
        ones_sb = consts.tile([1, BL], F32R, tag="ones")
        ones128 = consts.tile([128, 1], F32, tag="ones128")
        nc.vector.memset(ones128[:], 1.0)
        ones_row = consts.tile([1, 128], F32, tag="onesrow")
        nc.vector.memset(ones_row[:], 1.0)
        ve_f32 = consts.tile([128, 2], F32, tag="vef")
        nc.sync.dma_start(
            out=ve_f32[:],
            in_=bass.AP(tensor=ve_d, offset=0, ap=[[1, 128], [128, 2]]),
        )
        ve_sb = consts.tile([128, 2], BF16, tag="veb")
        nc.vector.tensor_copy(ve_sb[:], ve_f32[:])
        id_sb = consts.tile([128, 128], F32, tag="id")
        nc.sync.dma_start(out=id_sb[:], in_=id_d[:, :])
        idh_sb = consts.tile([128, 128], F32, tag="idh")
        nc.scalar.mul(idh_sb[:], id_sb[:], 0.5)

        # C storage: per t'-tile (128, 4096) bf16, free index = n*16 + b
        c_sb = consts.tile([128, 2, N * BL], BF16, tag="C")

        # ---- prologue: fp32r weight casts + C = X_perm @ WU_x^T ----
        with (
            tc.tile_pool(name="xsb", bufs=1) as xpool,
            tc.tile_pool(name="cps", bufs=4, space="PSUM") as cps,
        ):
            x_sb = xpool.tile([128, 2, BL * N], F32, tag="xsb")
            for kt in range(2):
                for b in range(BL):
                    nc.sync.dma_start(
                        out=x_sb[:, kt, b * N : (b + 1) * N],
                        in_=X_d[b, kt * 128 : (kt + 1) * 128, :],
                    )
            wux_sb = xpool.tile([128, 2 * T], F32R, tag="wux")
            wux_st = xpool.tile([128, 2 * T], F32, tag="wuxst")
            for kt in range(2):
                nc.sync.dma_start(
                    out=wux_st[:, kt * T : (kt + 1) * T],
                    in_=WUxT_d[kt * 128 : (kt + 1) * 128, :],
                )
            nc.vector.tensor_copy(wux_sb[:], wux_st[:])
            wst = xpool.tile([128, 2 * 4 * M], F32, tag="wst")
            for kt in range(2):
                nc.sync.dma_start(
                    out=wst[:, kt * 4 * M : (kt + 1) * 4 * M],
                    in_=WxT_d[kt * 128 : (kt + 1) * 128, :],
                )
            nc.vector.tensor_copy(wx_sb[:], wst[:])
            wst2 = xpool.tile([128, 2 * 4 * M], F32, tag="wst2")
            for kt in range(2):
                nc.sync.dma_start(
                    out=wst2[:, kt * 4 * M : (kt + 1) * 4 * M],
                    in_=WhT_d[kt * 128 : (kt + 1) * 128, :],
                )
            nc.vector.tensor_copy(wh_sb[:], wst2[:])
            bcst = xpool.tile([1, 4 * M], F32, tag="bcst")
            nc.sync.dma_start(out=bcst[:], in_=bc_d[:, :])
            nc.vector.tensor_copy(bc_sb[:], bcst[:])
            onest = xpool.tile([1, BL], F32, tag="onest")
            nc.vector.memset(onest[:], 1.0)
            nc.vector.tensor_copy(ones_sb[:], onest[:])

            # re-layout X to free = n*16 + b (matmul rhs must be 2D APs)
            x_re = xpool.tile([128, 2, BL * N], F32R, tag="xre")
            x_ap = x_sb[:]
            xr_ap = x_re[:]
            for kt in range(2):
                src = _bc_ap(x_ap, kt * BL * N, [[N, BL], [1, N]])
                dst = _bc_ap(xr_ap, kt * BL * N, [[1, BL], [BL, N]])
                nc.vector.tensor_copy(dst, src)
            for tt in range(2):
                for ch in range(8):  # 512-col chunks
                    cp = cps.tile([128, 512], F32, tag="cps")
                    for kt in range(2):
                        lhsT = wux_sb[:, kt * T + tt * 128 : kt * T + (tt + 1) * 128]
                        rhs = _bc_ap(xr_ap, kt * BL * N + ch * 512, [[1, 512]])
                        nc.tensor.matmul(
                            cp[:], lhsT, rhs, start=(kt == 0), stop=(kt == 1)
                        )
                    nc.vector.tensor_copy(c_sb[:, tt, ch * 512 : (ch + 1) * 512], cp[:])

        # ---- per-step pools ----
        pools = {
            "hst": ctx.enter_context(tc.tile_pool(name="hst", bufs=2)),
            "dpool": ctx.enter_context(tc.tile_pool(name="dpool", bufs=2)),
            "h2pool": ctx.enter_context(tc.tile_pool(name="h2", bufs=3)),
            "abf": ctx.enter_context(tc.tile_pool(name="abf", bufs=2)),
            "ppool": ctx.enter_context(tc.tile_pool(name="pp", bufs=2)),
            "ptpool": ctx.enter_context(tc.tile_pool(name="pt", bufs=2)),
            "xtp": ctx.enter_context(tc.tile_pool(name="xtp", bufs=4)),
            "sm": ctx.enter_context(tc.tile_pool(name="sm", bufs=2)),
            "gsb": ctx.enter_context(tc.tile_pool(name="gsb", bufs=2)),
            "gact": ctx.enter_context(tc.tile_pool(name="gact", bufs=2)),
            "obf": ctx.enter_context(tc.tile_pool(name="obf", bufs=4)),
            "aps_pool": ctx.enter_context(
                tc.tile_pool(name="aps", bufs=1, space="PSUM")
            ),
            "ets_pool": ctx.enter_context(
                tc.tile_pool(name="ets", bufs=1, space="PSUM")
            ),
            "ghb_pool": ctx.enter_context(
                tc.tile_pool(name="ghb", bufs=1, space="PSUM")
            ),
            "gx_pool": ctx.enter_context(tc.tile_pool(name="gx", bufs=1, space="PSUM")),
            "tps_pool": ctx.enter_context(
                tc.tile_pool(name="tps", bufs=1, space="PSUM")
            ),
            "otp_pool": ctx.enter_context(
                tc.tile_pool(name="otp", bufs=1, space="PSUM")
            ),
        }
        consts_d = {
            "c_ap": c_sb[:],
            "X_d": X_d,
            "outs_d": outs_d,
            "TQ": TQ,
            "scl_d": scl_d,
            "wuh_sb": wuh_sb,
            "wx_sb": wx_sb,
            "wh_sb": wh_sb,
            "bc_sb": bc_sb,
            "ones_sb": ones_sb,
            "ones128": ones128,
            "ones_row": ones_row,
            "ve_sb": ve_sb,
            "id_sb": id_sb,
            "idh_sb": idh_sb,
        }

        for rep in range(REPEAT):
            hsT = pools["hst"].tile([128, 4, BL], F32R, tag="hsT")
            nc.vector.memset(hsT[:].bitcast(F32), 0.0)
            d_prev = pools["dpool"].tile([128, 2, BL], F32, tag="D")
            nc.vector.memset(d_prev[:], 0.0)

            for t in range(TSTEPS):
                hsT, d_prev = step(nc, t, hsT, d_prev, pools, consts_d)

    nc.finalize()
    return nc


def step(nc, t, hsT, d_prev, pools, cd):
    """One recurrence step; returns hsT_new ([h2T | d2T] in [m, b] layout)."""
    c_ap = cd["c_ap"]
    X_d = cd["X_d"]
    TQ = cd["TQ"]
    out_d = cd["outs_d"][t // TQ]
    t_out = t % TQ

    # x_t prefetch
    x_t = pools["xtp"].tile([BL, N], F32, tag="xt")
    if "xdma" in SKIP:
        nc.vector.memset(x_t[:], 0.1)
    else:
        nc.sync.dma_start(out=x_t[:], in_=X_d[:, t, :])

    # trans scratch psum: [unused x4 | x_t^T x2 | sum | rec128]
    tr_ps = pools["tps_pool"].tile([128, 8, BL], F32, tag="trps")

    # gates bias+h part, transposed ([gate, b]); state-only deps; runs early
    g_hb = pools["ghb_pool"].tile([128, 8, BL], F32, tag="ghb")
    if "gates" in SKIP:
        nc.vector.memset(g_hb[:], 0.0)
    else:
        for gs in range(8):
            gsl = slice(gs * 128, (gs + 1) * 128)
            nc.tensor.matmul(
                g_hb[:, gs, :], cd["bc_sb"][:, gsl], cd["ones_sb"][:],
                start=True, stop=False,
            )
            for kt in range(2):
                wsl = slice(kt * 4 * M + gs * 128, kt * 4 * M + (gs + 1) * 128)
                nc.tensor.matmul(
                    g_hb[:, gs, :],
                    cd["wh_sb"][:, wsl],
                    hsT[:, kt, :],
                    start=False,
                    stop=(kt == 1),
                )
    g_hb_sb = pools["gsb"].tile([128, 8, BL], F32, tag="ghbsb")
    nc.vector.tensor_copy(g_hb_sb[:], g_hb[:])

    # A[t', b]
    a_ps = pools["aps_pool"].tile([128, 2, BL], F32, tag="aps")
    if "amm" in SKIP:
        nc.vector.memset(a_ps[:], 0.0)
    else:
        for tt in range(2):
            for kt in range(4):
                nc.tensor.matmul(
                    a_ps[:, tt, :],
                    cd["wuh_sb"][:, kt * T + tt * 128 : kt * T + (tt + 1) * 128],
                    hsT[:, kt, :].bitcast(F32),
                    start=(kt == 0),
                    stop=(kt == 3),
                )
    a_bf = pools["abf"].tile([128, 2, BL], BF16, tag="abf")
    nc.vector.tensor_copy(a_bf[:], a_ps[:])
    a_ap = a_bf[:]

    # P = tanh(C + A)
    p_pre = pools["ppool"].tile([128, 2, N * BL], BF16, tag="ppre")
    p_tanh = pools["ptpool"].tile([128, 2, N * BL], BF16, tag="ptanh")
    pp_ap = p_pre[:]
    pt_ap = p_tanh[:]
    if "add" in SKIP:
        nc.vector.memset(p_pre[:].bitcast(U16), 0)
    if "tanh" in SKIP:
        nc.vector.memset(p_tanh[:].bitcast(U16), 0)
    for tt in range(2):
        for half in range(2):
            b0 = half * 8
            dims = [[BL, N], [1, 8]]
            in0 = _bc_ap(c_ap, tt * N * BL + b0, dims)
            o0 = _bc_ap(pp_ap, tt * N * BL + b0, dims)
            o1 = _bc_ap(pt_ap, tt * N * BL + b0, dims)
            a_in = _bc_ap(a_ap, tt * BL + b0, [[0, N], [1, 8]])
            if "add" not in SKIP:
                nc.vector.tensor_tensor(o0, in0, a_in, ALU.add)
            if "tanh" not in SKIP:
                nc.scalar.activation(o1, o0, AF.Tanh)

    # e^T[n, b] = sum_t' P[t', n, b] * ve[t']
    et_ps = pools["ets_pool"].tile([128, 2, BL], F32, tag="etps")
    if "etmm" in SKIP:
        nc.vector.memset(et_ps[:], 1.0)
    else:
        for nsl in range(2):
            for b in range(BL):
                for tt in range(2):
                    lhsT = _bc_ap(
                        pt_ap, tt * N * BL + nsl * 128 * BL + b, [[BL, 128]]
                    )
                    nc.tensor.matmul(
                        et_ps[:, nsl, b : b + 1],
                        lhsT,
                        cd["ve_sb"][:, tt : tt + 1],
                        start=(tt == 0),
                        stop=(tt == 1),
                    )

    hsT_new = pools["hst"].tile([128, 4, BL], F32R, tag="hsT")
    d_new = pools["dpool"].tile([128, 2, BL], F32, tag="D")
    h2t = pools["h2pool"].tile([128, 2, BL], F32, tag="H2")
    if "small" in SKIP:
        nc.vector.memset(hsT_new[:].bitcast(F32), 0.0)
        nc.vector.memset(d_new[:], 0.0)
        nc.vector.memset(h2t[:], 0.0)
    else:
        # softmax over n (transposed); exp then sum via ones-matmul
        exp_t = pools["sm"].tile([128, 2, BL], F32, tag="expT")
        nc.scalar.activation(exp_t[:], et_ps[:], AF.Exp)
        for nsl in range(2):
            nc.tensor.matmul(
                tr_ps[0:1, 6, :],
                cd["ones128"][:],
                exp_t[:, nsl, :],
                start=(nsl == 0),
                stop=(nsl == 1),
            )
        rec_row = pools["sm"].tile([1, BL], F32, tag="recrow")
        nc.vector.reciprocal(rec_row[:], tr_ps[0:1, 6, :])
        # broadcast 1/sum over gate partitions: outer(ones128, rec_row)
        nc.tensor.matmul(
            tr_ps[:, 7, :], cd["ones_row"][:], rec_row[:], start=True, stop=True
        )

        # xu^T = exp^T * x_t^T (unnormalized x_tilde, transposed)
        for kt in range(2):
            nc.tensor.transpose(
                tr_ps[:, 4 + kt, :],
                x_t[:, kt * 128 : (kt + 1) * 128],
                cd["id_sb"][0:BL, 0:BL],
            )
        xu = pools["sm"].tile([128, 2, BL], F32R, tag="xu")
        nc.vector.tensor_tensor(xu[:], exp_t[:], tr_ps[:, 4:6, :], ALU.mult)

        # gates x-part, transposed ([gate, b])
        g_x = pools["gx_pool"].tile([128, 8, BL], F32, tag="gx")
        if "gates" in SKIP:
            nc.vector.memset(g_x[:], 0.0)
        else:
            for gs in range(8):
                for kt in range(2):
                    wsl = slice(kt * 4 * M + gs * 128, kt * 4 * M + (gs + 1) * 128)
                    nc.tensor.matmul(
                        g_x[:, gs, :],
                        cd["wx_sb"][:, wsl],
                        xu[:, kt, :],
                        start=(kt == 0),
                        stop=(kt == 1),
                    )

        # combined gates (order [i f o g] along the 8 gate tiles)
        rec_sb = pools["sm"].tile([128, BL], F32, tag="recsb")
        nc.vector.tensor_copy(rec_sb[:], tr_ps[:, 7, :])
        g1 = pools["gsb"].tile([128, 8, BL], F32, tag="g1")
        rec_bc = _bc_ap(rec_sb[:], 0, [[0, 8], [1, BL]])
        nc.vector.tensor_tensor(g1[:], g_x[:], rec_bc, ALU.mult)
        gc = pools["gsb"].tile([128, 8, BL], F32, tag="gc")
        nc.vector.tensor_tensor(gc[:], g1[:], g_hb_sb[:], ALU.add)
        t_ifo = pools["gact"].tile([128, 6, BL], F32, tag="tifo")
        t_g = pools["gact"].tile([128, 2, BL], F32, tag="tg")
        nc.scalar.activation(t_ifo[:], gc[:, 0:6, :], AF.Tanh, scale=0.5)
        nc.scalar.activation(t_g[:], gc[:, 6:8, :], AF.Tanh)

        # D_new = (t_f+1)*D/2 + (t_i+1)*t_g ; H2 = (t_o+1)*tanh(D_new/2)
        u = pools["gact"].tile([128, 2, BL], F32, tag="u")
        v = pools["gact"].tile([128, 2, BL], F32, tag="v")
        nc.vector.scalar_tensor_tensor(
            u[:], t_ifo[:, 2:4, :], 1.0, d_prev[:], ALU.add, ALU.mult
        )
        nc.vector.scalar_tensor_tensor(
            v[:], t_ifo[:, 0:2, :], 1.0, t_g[:], ALU.add, ALU.mult
        )
        nc.vector.scalar_tensor_tensor(d_new[:], u[:], 0.5, v[:], ALU.mult, ALU.add)
        tanh_c = pools["gact"].tile([128, 2, BL], F32, tag="tc")
        nc.scalar.activation(tanh_c[:], d_new[:], AF.Tanh, scale=0.5)
        nc.vector.scalar_tensor_tensor(
            h2t[:], t_ifo[:, 4:6, :], 1.0, tanh_c[:], ALU.add, ALU.mult
        )
        # rounded fp32r copies for next step's matmuls
        nc.vector.tensor_copy(hsT_new[:, 0:2, :], h2t[:])
        nc.vector.tensor_copy(hsT_new[:, 2:4, :], d_new[:])

    # store output: transpose h2^T to [b, m], row-quantize to int8 with a
    # per-row abs-max scale (q = h2 * 126.5/mx; scl = mx/253 so h = q*scl)
    if "odma" not in SKIP:
        otp = pools["otp_pool"].tile([128, M], F32, tag="otp")
        for kt in range(2):
            nc.tensor.transpose(
                otp[0:BL, kt * 128 : (kt + 1) * 128],
                h2t[:, kt, :],
                cd["id_sb"][:],
            )
        mx = pools["obf"].tile([128, 1], F32, tag="mx")
        nc.vector.tensor_reduce(
            mx[0:BL, :], otp[0:BL, :], axis=mybir.AxisListType.X,
            op=ALU.max, apply_absolute_value=True,
        )
        # guard all-zero rows (h==0): max with tiny epsilon
        mxe = pools["obf"].tile([128, 1], F32, tag="mxe")
        nc.vector.tensor_scalar_max(mxe[0:BL, :], mx[0:BL, :], 1e-30)
        rq = pools["obf"].tile([128, 1], F32, tag="rq")
        nc.vector.reciprocal(rq[0:BL, :], mxe[0:BL, :])
        qi8 = pools["obf"].tile([BL, M], mybir.dt.int8, tag="qi8")
        nc.vector.tensor_scalar(
            qi8[:], otp[0:BL, :], rq[0:BL, :], 126.5, ALU.mult, ALU.mult
        )
        scl = pools["obf"].tile([128, 1], F32, tag="scl")
        nc.vector.tensor_scalar_mul(scl[0:BL, :], mxe[0:BL, :], 1.0 / 253.0)
        nc.sync.dma_start(out=out_d[t_out, :, :], in_=qi8[:])
        nc.sync.dma_start(
            out=bass.AP(tensor=cd["scl_d"], offset=t * BL, ap=[[1, BL], [1, 1]]),
            in_=scl[0:BL, :],
        )

    return hsT_new, d_new


_PROGRAM = None


def _get_program():
    global _PROGRAM
    if _PROGRAM is None:
        _PROGRAM = build_program()
    return _PROGRAM


def _preprocess(WU_e, v_e, W_ih, W_hh, b_ih, b_hh):
    """Host-side weight refactors (fold 0.5 for the sigmoid-as-tanh trick)."""
    m = M
    WUhT = np.ascontiguousarray((WU_e[:, : 2 * m] * 0.5).T)  # (2M, T)
    WUxT = np.ascontiguousarray(WU_e[:, 2 * m :].T)  # (T, T)

    def reorder(w):
        i, f, g, o = np.split(w, 4, axis=0)
        return np.concatenate([i, f, o, g], axis=0)

    WxT = np.ascontiguousarray(reorder(W_ih).T)  # (N, 4M)
    WhT = np.ascontiguousarray((reorder(W_hh) * 0.5).T)  # (M, 4M)
    bc = np.ascontiguousarray(reorder(b_ih + b_hh)[None, :])  # (1, 4M)
    ve = np.ascontiguousarray(v_e[0][:, None])  # (T, 1)
    ident = np.eye(128, dtype=np.float32)
    return {
        "WUxT": WUxT,
        "WUhT": WUhT,
        "WxT": WxT,
        "WhT": WhT,
        "bc": bc,
        "ve": ve,
        "ident": ident,
    }


class _Runner:
    """AOT-compiled dispatcher over the same PJRT/bass_exec path that
    run_bass_kernel_spmd uses under axon, but with the jit traced, lowered
    and compiled exactly once per process, weights cached on-device, and
    donated output buffers created on-device (no host zero upload)."""

    def __init__(self):
        import jax
        import jax.numpy as jnp
        from jax.experimental.shard_map import shard_map
        from jax.sharding import Mesh, NamedSharding, PartitionSpec

        import concourse.bass2jax as b2j

        self.jax = jax
        nc = _get_program()
        b2j.install_neuronx_cc_hook()

        pname = (
            nc.partition_id_tensor.name
            if nc.partition_id_tensor is not None
            else None
        )
        self.dbg_name = nc.dbg_addr.name if nc.dbg_addr is not None else None
        if self.dbg_name is not None and nc.dbg_callbacks:
            raise RuntimeError("dbg callbacks unsupported in fast path")

        in_names, out_names, out_avals, in_shapes = [], [], [], {}
        for alloc in nc.m.functions[0].allocations:
            if not isinstance(alloc, mybir.MemoryLocationSet):
                continue
            name = alloc.memorylocations[0].name
            if alloc.kind == "ExternalInput":
                if name != pname:
                    in_names.append(name)
                    in_shapes[name] = (
                        tuple(alloc.tensor_shape),
                        mybir.dt.np(alloc.dtype),
                    )
            elif alloc.kind == "ExternalOutput":
                out_names.append(name)
                out_avals.append(
                    jax.core.ShapedArray(
                        tuple(alloc.tensor_shape), mybir.dt.np(alloc.dtype)
                    )
                )
        if self.dbg_name is not None and self.dbg_name not in in_names:
            in_names.append(self.dbg_name)
            in_shapes[self.dbg_name] = ((1, 2), np.uint32)
        self.in_names = in_names
        self.out_names = out_names
        self.out_avals = out_avals

        n_params = len(in_names)
        n_outs = len(out_names)
        all_in_names = list(in_names) + list(out_names)
        if pname is not None:
            all_in_names.append(pname)
        donate = tuple(range(n_params, n_params + n_outs))

        def _body(*args):
            operands = list(args)
            if pname is not None:
                operands.append(b2j.partition_id_tensor())
            outs = b2j._bass_exec_p.bind(
                *operands,
                out_avals=tuple(out_avals),
                in_names=tuple(all_in_names),
                out_names=tuple(out_names),
                lowering_input_output_aliases=(),
                sim_require_finite=True,
                sim_require_nnan=True,
                nc=nc,
            )
            return tuple(outs)

        devices = jax.devices()[:NCORES]
        assert len(devices) == NCORES
        mesh = Mesh(np.asarray(devices), ("core",))
        self.sharding = NamedSharding(mesh, PartitionSpec("core"))
        in_specs = (PartitionSpec("core"),) * (n_params + n_outs)
        out_specs = (PartitionSpec("core"),) * n_outs

        def g_sds(shape, dtype):
            return jax.ShapeDtypeStruct(
                (NCORES * shape[0], *shape[1:]), dtype, sharding=self.sharding
            )

        in_sds = [g_sds(*in_shapes[n]) for n in in_names]
        out_sds = [g_sds(a.shape, a.dtype) for a in out_avals]

        self.compiled = b2j.fast_dispatch_compile(
            lambda: jax.jit(
                shard_map(
                    _body,
                    mesh=mesh,
                    in_specs=in_specs,
                    out_specs=out_specs,
                    check_rep=False,
                ),
                donate_argnums=donate,
                keep_unused=True,
            )
            .lower(*in_sds, *out_sds)
            .compile()
        )
        self.zeros_fn = jax.jit(
            lambda: tuple(
                jnp.zeros((NCORES * a.shape[0], *a.shape[1:]), a.dtype)
                for a in out_avals
            ),
            out_shardings=tuple(self.sharding for _ in out_avals),
        )
        from concurrent.futures import ThreadPoolExecutor

        self.pool = ThreadPoolExecutor(9)
        self.wkey = None
        self.wdev = None
        self.xkey = None
        self.xdev = None
        self.donate_next = None  # previous call's output, recycled as buffer

    @staticmethod
    def _ckey(*arrs):
        h = 0
        for a in arrs:
            b = np.ascontiguousarray(a).view(np.uint8)
            h = zlib.crc32(b, h)
        return h

    def run(self, X, WU_e, v_e, W_ih, W_hh, b_ih, b_hh):
        jax = self.jax
        t0 = time.time()
        wkey = self._ckey(WU_e, v_e, W_ih, W_hh, b_ih, b_hh)
        if wkey != self.wkey:
            host = _preprocess(WU_e, v_e, W_ih, W_hh, b_ih, b_hh)
            if self.dbg_name is not None:
                host[self.dbg_name] = np.zeros((1, 2), np.uint32)
            self.wdev = {}
            for name in self.in_names:
                if name == "X":
                    continue
                v = host[name]
                tiled = np.ascontiguousarray(
                    np.broadcast_to(v, (NCORES, *v.shape))
                ).reshape(NCORES * v.shape[0], *v.shape[1:])
                self.wdev[name] = jax.device_put(tiled, self.sharding)
            jax.block_until_ready(list(self.wdev.values()))
            self.wkey = wkey
        t1 = time.time()
        # The kernel overwrites every element of out each call, so the
        # donated buffer's contents are irrelevant: recycle the previous
        # call's (already host-fetched) output instead of making zeros.
        donate = self.donate_next
        if donate is None:
            donate = self.zeros_fn()
        if self.xdev is None:
            # first call: upload X before submitting
            self.xkey = self._ckey(X)
            self.xdev = jax.device_put(X, self.sharding)
            speculated = False
        else:
            speculated = True  # submit with cached X; verify hash in parallel
        dev_in = [self.xdev if n == "X" else self.wdev[n] for n in self.in_names]
        try:
            outs = self.compiled(*dev_in, *donate)
            # start the D2H pulls immediately; they block (GIL released)
            # until the remote exec completes, so the X-hash below and the
            # fetch initiation both overlap the execution
            futs = [self.pool.submit(np.asarray, o) for o in outs]
            if speculated:
                xkey = self._ckey(X)  # overlaps the remote execution
                if xkey != self.xkey:
                    # X changed: drain the stale fetches, then redo with
                    # the real X, recycling the speculative outputs as
                    # donated buffers
                    for f in futs:
                        f.result()
                    self.xkey = xkey
                    self.xdev = jax.device_put(X, self.sharding)
                    dev_in = [
                        self.xdev if n == "X" else self.wdev[n]
                        for n in self.in_names
                    ]
                    outs = self.compiled(*dev_in, *outs)
                    futs = [self.pool.submit(np.asarray, o) for o in outs]
        except Exception:
            self.donate_next = None
            raise
        self.donate_next = outs
        t2 = t3 = time.time()
        # dequantize each payload quarter as it lands; later quarters
        # stream while earlier ones multiply
        NSPLIT = len(outs) - 1
        TQ = (TSTEPS + NSPLIT - 1) // NSPLIT
        scl = futs[-1].result()
        s4 = scl.reshape(NCORES, TSTEPS, BL, 1).transpose(1, 0, 2, 3)
        full = np.empty((TSTEPS, NCORES, BL, M), np.float32)
        for p in range(NSPLIT):
            t0p = p * TQ
            tp = min(TQ, TSTEPS - t0p)
            q = futs[p].result()
            np.multiply(
                q.reshape(NCORES, tp, BL, M).transpose(1, 0, 2, 3),
                s4[t0p : t0p + tp],
                out=full[t0p : t0p + tp],
            )
        t4 = time.time()
        full = full.reshape(TSTEPS, B, M)
        t5 = time.time()
        if DEBUG_TIMING:
            print(
                f"[kernel] wput {t1 - t0:.3f}s xput {t2 - t1:.3f}s "
                f"exec {t3 - t2:.3f}s d2h {t4 - t3:.3f}s host {t5 - t4:.3f}s"
            )
        return full


_RUNNER = None
_RUNNER_FAILED = False


def _get_runner():
    global _RUNNER, _RUNNER_FAILED
    if _RUNNER is None and not _RUNNER_FAILED:
        try:
            _RUNNER = _Runner()
        except Exception as e:  # fall back to the stock dispatch path
            import traceback

            traceback.print_exc()
            print(f"[kernel] fast path unavailable ({e!r}); using spmd fallback")
            _RUNNER_FAILED = True
    return _RUNNER


def kernel(X, WU_e, v_e, W_ih, W_hh, b_ih, b_hh):
    X = np.ascontiguousarray(X, dtype=np.float32)
    WU_e = np.asarray(WU_e, dtype=np.float32)
    v_e = np.asarray(v_e, dtype=np.float32)
    W_ih = np.asarray(W_ih, dtype=np.float32)
    W_hh = np.asarray(W_hh, dtype=np.float32)
    b_ih = np.asarray(b_ih, dtype=np.float32)
    b_hh = np.asarray(b_hh, dtype=np.float32)

    runner = _get_runner()
    if runner is not None:
        try:
            return runner.run(X, WU_e, v_e, W_ih, W_hh, b_ih, b_hh).astype(
                np.float32
            )
        except Exception:
            import traceback

            traceback.print_exc()
            print("[kernel] fast path failed at runtime; using spmd fallback")
            global _RUNNER, _RUNNER_FAILED
            _RUNNER = None
            _RUNNER_FAILED = True

    host = _preprocess(WU_e, v_e, W_ih, W_hh, b_ih, b_hh)
    nc = _get_program()
    in_maps = []
    for c in range(NCORES):
        in_maps.append(
            {"X": np.ascontiguousarray(X[c * BL : (c + 1) * BL]), **host}
        )
    res = run_bass_kernel_spmd(nc, in_maps, list(range(NCORES)))
    parts = []
    nsplit = len([k for k in res.results[0] if k.startswith("out")])
    for i in range(NCORES):
        q = np.concatenate(
            [res.results[i][f"out{p}"] for p in range(nsplit)], axis=0
        )  # (T, BL, M) int8
        s = res.results[i]["scl"]  # (T, BL) f32
        parts.append(np.multiply(q, s[:, :, None], dtype=np.float32))
    return np.concatenate(parts, axis=1).astype(np.float32)

